# revision 11
# baseline (speedup 1.0000x reference)
"""LoRA linear (y = x @ (W + s*B@A)^T + bias) on 8 Trainium2 NeuronCores.

Strategy: pure data parallel over the token dim. The LoRA update is folded
into the weight on the host (W' = W + 4.0 * B @ A, rank-8 update), and the
bias is added on the host after the device matmul, so the device kernel is a
pure GEMM. All matmul operands are bf16 (fro rel-err ~2.9e-3 vs the 2e-2
gate): halves HBM traffic vs fp32 (20.9 -> 10.5 MB/core) and enables the PE
fast-weight-load path (FWL, non-fp32 only). PSUM accumulation stays fp32;
the output is stored bf16 and upcast to fp32 on the host.

Per core: out[2048, 1024] = xT[:, shard].T @ wT
  - wT [1024(d), 1024(o)] resident in SBUF (2 MiB), loaded once, streamed on
    the scalar-engine HWDGE ring (qActDynamicHW)
  - x resident as 8 d-tiles x 2 super-blocks of 1024 tokens, streamed on the
    sync-engine HWDGE ring (qSPDynamicHW) in exact consumption order; the
    two rings run concurrently so the startup fill is ~2x faster
  - psum [128(n), 1024(o)] accumulated over 8 d-tiles, 2 o-halves of 512;
    4 psum tiles (all 8 banks) accumulate side by side so each arriving
    (w[d], x[d]) slice enables 8 matmuls during the fill phase
  - d=7 (the accumulation-closing matmuls) is issued per-psum-tile with the
    eviction immediately after, so the 4 evictions of a group stagger
    across the tail of the group instead of bunching at the boundary
  - evictions are pure fp32->bf16 copies, alternating between the scalar
    (ACT) and vector (DVE) engines so two run concurrently
  - out DMAs ride the scalar ring (empty after the w fill)
"""

import os
import sys

import numpy as np

for _p in ("/opt/trn_rl_repo", "/opt/pypackages"):
    if os.path.isdir(_p) and _p not in sys.path:
        sys.path.append(_p)

try:
    import jax

    jax.config.update(
        "jax_compilation_cache_dir", os.path.expanduser("~/.cache/jax_bass_cache")
    )
    jax.config.update("jax_persistent_cache_min_compile_time_secs", 0.0)
except Exception:
    pass

try:
    # bass_utils imports this when tracing is requested via BASS_TRACE; the
    # agent image ships a stub antenv without it. Register a no-op fallback
    # so a trace request degrades to "no trace" instead of crashing.
    from antenv import axon_hooks as _axon_hooks  # noqa: F401
except ImportError:
    import types as _types

    import antenv as _antenv

    _hooks = _types.ModuleType("antenv.axon_hooks")
    _hooks._hook = None
    _hooks.set_axon_ntff_profile_hook = lambda h: setattr(_hooks, "_hook", h)
    _hooks.get_axon_ntff_profile_hook = lambda: _hooks._hook
    sys.modules["antenv.axon_hooks"] = _hooks
    _antenv.axon_hooks = _hooks

import ml_dtypes  # noqa: E402

import concourse.bass as bass  # noqa: E402,F401
import concourse.mybir as mybir  # noqa: E402
import concourse.tile as tile  # noqa: E402
from concourse import bacc  # noqa: E402
from concourse.bass_utils import run_bass_kernel_spmd  # noqa: E402

N_CORES = 8
N_TOK, D_IN, D_OUT = 16384, 1024, 1024
N_SHARD = N_TOK // N_CORES  # 2048 tokens per core
P = 128
SCALING = 4.0  # alpha / r = 32 / 8

BF16 = ml_dtypes.bfloat16

_CACHE: dict = {}


def build_nc():
    f32 = mybir.dt.float32
    bf16 = mybir.dt.bfloat16
    nc = bacc.Bacc("TRN2", target_bir_lowering=False, debug=False)

    xT = nc.dram_tensor("xT", [D_IN, N_SHARD], bf16, kind="ExternalInput")
    wT = nc.dram_tensor("wT", [D_IN, D_OUT], bf16, kind="ExternalInput")
    out = nc.dram_tensor("out", [N_SHARD, D_OUT], bf16, kind="ExternalOutput")

    KT = D_IN // P  # 8 contraction tiles
    NBLK = 512  # tokens per group (4 psum tiles of 128)
    GRP = NBLK // P  # 4 psum tiles accumulated concurrently (8 banks)
    OH = 512  # max fp32 moving free dim (one PSUM bank)

    NGRP = N_SHARD // NBLK
    XBLK = 2 * NBLK  # 1024 tokens per x tile, serves 2 groups
    with tile.TileContext(nc) as tc:
        with tc.tile_pool(name="const", bufs=1) as const_pool, \
                tc.tile_pool(name="xp", bufs=2 * KT) as x_pool, \
                tc.tile_pool(name="op", bufs=8) as out_pool, \
                tc.tile_pool(name="ps", bufs=GRP, space="PSUM") as psum_pool:
            w_tiles = [
                const_pool.tile([P, D_OUT], bf16, name=f"w{t}")
                for t in range(KT)
            ]

            # Leading warm-up matmuls (zeroed bf16 scratch via the vector
            # engine, which starts earliest): the SDMA rings only begin
            # streaming ~2.3us after the sequencers start, so real operands
            # cannot arrive before ~8.3us. Cold N=256 matmuls from ~6.9us
            # fill the HAM clock-gate's 3.4us busy window so the real matmul
            # stream starts (nearly) at the warm 2.4 GHz clock; N=256 keeps
            # the granularity fine so the last warm-up barely delays the
            # first real matmul.
            warm_x = const_pool.tile([P, P], bf16)
            warm_w = const_pool.tile([P, OH // 2], bf16)
            nc.vector.memset(warm_x[:], 0.0)
            nc.vector.memset(warm_w[:], 0.0)
            warm_ps = psum_pool.tile([P, OH], f32, name="warm_ps", tag="psum")
            for _ in range(7):
                nc.tensor.matmul(warm_ps[:, 0:OH // 2], warm_x[:], warm_w[:],
                                 start=True, stop=True)

            # Startup streams in exact consumption order, on two concurrent
            # HWDGE rings. The first-needed chunks (x0 quarter, then all of
            # w0) ride the sync ring, which kicks off ~0.6us before the
            # scalar ring; the scalar ring brings w1..w7 concurrently. Tile
            # dependencies are sub-tile-range granular, so x tiles load in
            # halves: group 0 only needs token-half 0 of each d-slice; half 1
            # (group 1) streams after.
            x_super = {}
            xs = [
                [
                    x_pool.tile([P, XBLK], bf16, name=f"x_gg{gg}_d{t}",
                                tag="xd")
                    for t in range(KT)
                ]
                for gg in range(NGRP // 2)
            ]
            x_super[0] = xs[0]
            x_super[1] = xs[1]
            # Group 0 runs as two o-half passes (see below), so the fill only
            # needs the h0 half of each w tile early: 256KB per d-step
            # (128KB w-h0 + 128KB x) matches the ring's ramp-up rate. The
            # first-needed chunks (x0 quarter, w0-h0) ride the sync ring,
            # which kicks off first; w1..w7 h0-halves then all h1-halves ride
            # the scalar ring concurrently.
            nc.sync.dma_start(xs[0][0][:, 0:P], xT[0:P, 0:P])
            nc.sync.dma_start(w_tiles[0][:, 0:OH], wT[0:P, 0:OH])
            for t in range(1, KT):
                nc.scalar.dma_start(w_tiles[t][:, 0:OH],
                                    wT[t * P:(t + 1) * P, 0:OH])
            nc.scalar.dma_start(w_tiles[0][:, OH:D_OUT], wT[0:P, OH:D_OUT])
            for t in range(1, KT):
                nc.scalar.dma_start(w_tiles[t][:, OH:D_OUT],
                                    wT[t * P:(t + 1) * P, OH:D_OUT])

            nc.sync.dma_start(xs[0][0][:, P:NBLK], xT[0:P, P:NBLK])
            for t in range(1, KT):
                nc.sync.dma_start(
                    xs[0][t][:, 0:NBLK], xT[t * P:(t + 1) * P, 0:NBLK]
                )
            for t in range(KT):
                nc.sync.dma_start(
                    xs[0][t][:, NBLK:XBLK], xT[t * P:(t + 1) * P, NBLK:XBLK]
                )
            for t in range(KT):
                nc.sync.dma_start(
                    xs[1][t][:], xT[t * P:(t + 1) * P, XBLK:2 * XBLK]
                )

            def evict(g, i, psum, split=False):
                # Pure psum->sbuf copy with fp32->bf16 convert; bias is added
                # on the host. Alternate engines so two evictions overlap.
                n0 = g * NBLK + i * P
                o_sb = out_pool.tile([P, D_OUT], bf16)
                if split:
                    # tail: halves on both engines concurrently, DMA per half
                    # on both rings.
                    nc.scalar.copy(o_sb[:, 0:OH], psum[:, 0:OH])
                    nc.vector.tensor_copy(o_sb[:, OH:D_OUT], psum[:, OH:D_OUT])
                    nc.scalar.dma_start(out[n0:n0 + P, 0:OH], o_sb[:, 0:OH])
                    nc.sync.dma_start(out[n0:n0 + P, OH:D_OUT],
                                      o_sb[:, OH:D_OUT])
                else:
                    if i % 2 == 0:
                        nc.scalar.copy(o_sb[:], psum[:])
                    else:
                        nc.vector.tensor_copy(o_sb[:], psum[:])
                    nc.scalar.dma_start(out[n0:n0 + P, :], o_sb[:])

            for g in range(NGRP):
                gg, half = divmod(g, 2)
                xt = [
                    x_super[gg][t][:, half * NBLK:(half + 1) * NBLK]
                    for t in range(KT)
                ]
                psums = [
                    psum_pool.tile([P, D_OUT], f32, name=f"ps_g{g}_{i}",
                                   tag="psum")
                    for i in range(GRP)
                ]
                if g == 0:
                    # Fill phase: two o-half passes. Pass h only touches
                    # w[:, h*512:(h+1)*512], so each d-step needs 256KB from
                    # HBM instead of 384KB — matching the SDMA rings' ramp-up
                    # rate so the PE doesn't starve. Each (i, h) psum bank
                    # closes at its d=7 matmul and evicts immediately; h0
                    # evictions overlap the h1 pass (different PSUM banks).
                    o_sbs = [out_pool.tile([P, D_OUT], bf16, name=f"og0_{i}")
                             for i in range(GRP)]
                    for h in range(D_OUT // OH):
                        sl = slice(h * OH, (h + 1) * OH)
                        for d in range(KT):
                            for i in range(GRP):
                                lhsT = xt[d][:, i * P:(i + 1) * P]
                                nc.tensor.matmul(
                                    psums[i][:, sl],
                                    lhsT,
                                    w_tiles[d][:, sl],
                                    start=(d == 0),
                                    stop=(d == KT - 1),
                                )
                                if d == KT - 1:
                                    n0 = g * NBLK + i * P
                                    if (i + h) % 2 == 0:
                                        nc.scalar.copy(o_sbs[i][:, sl],
                                                       psums[i][:, sl])
                                    else:
                                        nc.vector.tensor_copy(
                                            o_sbs[i][:, sl], psums[i][:, sl])
                                    nc.scalar.dma_start(out[n0:n0 + P, sl],
                                                        o_sbs[i][:, sl])
                elif g < NGRP - 1:
                    # d-outer for d<7: each arriving (w[d], x[d]) slice
                    # immediately enables 8 matmuls while later slices are in
                    # flight. d=7 goes per-psum-tile with the eviction right
                    # after, staggering evictions across the group tail.
                    for d in range(KT - 1):
                        for i in range(GRP):
                            lhsT = xt[d][:, i * P:(i + 1) * P]
                            for h in range(D_OUT // OH):
                                nc.tensor.matmul(
                                    psums[i][:, h * OH:(h + 1) * OH],
                                    lhsT,
                                    w_tiles[d][:, h * OH:(h + 1) * OH],
                                    start=(d == 0),
                                    stop=False,
                                )
                    d = KT - 1
                    for i in range(GRP):
                        lhsT = xt[d][:, i * P:(i + 1) * P]
                        for h in range(D_OUT // OH):
                            nc.tensor.matmul(
                                psums[i][:, h * OH:(h + 1) * OH],
                                lhsT,
                                w_tiles[d][:, h * OH:(h + 1) * OH],
                                start=False,
                                stop=True,
                            )
                        evict(g, i, psums[i])
                else:
                    # last group, data resident: i-outer spreads psum
                    # completions so the tail isn't 4 serialized evictions.
                    for i in range(GRP):
                        for d in range(KT):
                            lhsT = xt[d][:, i * P:(i + 1) * P]
                            for h in range(D_OUT // OH):
                                nc.tensor.matmul(
                                    psums[i][:, h * OH:(h + 1) * OH],
                                    lhsT,
                                    w_tiles[d][:, h * OH:(h + 1) * OH],
                                    start=(d == 0),
                                    stop=(d == KT - 1),
                                )
                        evict(g, i, psums[i], split=True)

    nc.finalize()
    return nc


def _get_nc():
    if "nc" not in _CACHE:
        _CACHE["nc"] = build_nc()
    return _CACHE["nc"]


def kernel(x, weight, bias, A, B):
    x = np.asarray(x, dtype=np.float32)
    weight = np.asarray(weight, dtype=np.float32)
    bias = np.asarray(bias, dtype=np.float32)
    A = np.asarray(A, dtype=np.float32)
    B = np.asarray(B, dtype=np.float32)

    # Fold the rank-8 LoRA update into the weight (exact up to fp32 rounding).
    w_eff = (
        weight.astype(np.float64) + SCALING * (B.astype(np.float64) @ A.astype(np.float64))
    ).astype(np.float32)
    wT = np.ascontiguousarray(w_eff.T).astype(BF16)  # [d, o]
    xT = np.ascontiguousarray(x.T).astype(BF16)  # [d, n]

    nc = _get_nc()
    in_maps = [
        {
            "xT": np.ascontiguousarray(xT[:, c * N_SHARD:(c + 1) * N_SHARD]),
            "wT": wT,
        }
        for c in range(N_CORES)
    ]
    trace_kwargs = {}
    if os.environ.get("KERNEL_TRACE") == "1":
        trace_kwargs = {"trace": True}
    res = run_bass_kernel_spmd(nc, in_maps, list(range(N_CORES)), **trace_kwargs)
    _CACHE["last_results"] = res
    y = np.concatenate(
        [r["out"] for r in res.results], axis=0
    ).astype(np.float32)
    y += bias[None, :]
    return y


# revision 12
# speedup vs baseline: 1.0572x; 1.0572x over previous
"""LoRA linear (y = x @ (W + s*B@A)^T + bias) on 8 Trainium2 NeuronCores.

Strategy: pure data parallel over the token dim. The LoRA update is folded
into the weight on the host (W' = W + 4.0 * B @ A, rank-8 update), and the
bias is added on the host after the device matmul, so the device kernel is a
pure GEMM. All matmul operands are bf16 (fro rel-err ~2.9e-3 vs the 2e-2
gate): halves HBM traffic vs fp32 (20.9 -> 10.5 MB/core) and lets the PE
hide LDWEIGHTS entirely (measured: warm N=512 matmuls issue back-to-back at
216ns with the per-matmul weight load overlapped, i.e. the PE runs at its
streaming roofline). PSUM accumulation stays fp32; the output is stored bf16
and upcast to fp32 on the host.

Per core: out[2048, 1024] = xT[:, shard].T @ wT
  - wT [1024(d), 1024(o)] resident in SBUF (2 MiB), loaded once on the
    scalar-engine HWDGE ring; x streams on the sync-engine ring in exact
    consumption order (the two rings run concurrently)
  - the SDMA rings only start streaming ~2.2us after the sequencers start,
    so real operands land from ~8.2us; six warm-up matmuls on zeroed scratch
    (memset by the vector engine, which starts earliest) occupy the PE's
    HAM clock-gate window from ~7us so the real matmul stream runs at the
    warm 2.4 GHz clock almost immediately
  - psum [128(n), 1024(o)] accumulated over 8 d-tiles, 2 o-halves of 512;
    4 psum tiles (all 8 banks) accumulate side by side so each arriving
    (w[d], x[d]) slice enables 8 matmuls during the fill phase
  - d=7 (the accumulation-closing matmuls) is issued per-psum-tile with the
    eviction immediately after, so the 4 evictions of a group stagger
    across the group tail instead of bunching at the boundary
  - evictions are pure fp32->bf16 copies alternating between the scalar
    (ACT) and vector (DVE) engines so two run concurrently; out DMAs ride
    the scalar ring (empty after the w fill), and the last group closes
    i-at-a-time with half-evictions on both engines and half-DMAs on both
    rings to shorten the tail
"""

import os
import sys

import numpy as np

for _p in ("/opt/trn_rl_repo", "/opt/pypackages"):
    if os.path.isdir(_p) and _p not in sys.path:
        sys.path.append(_p)

try:
    import jax

    jax.config.update(
        "jax_compilation_cache_dir", os.path.expanduser("~/.cache/jax_bass_cache")
    )
    jax.config.update("jax_persistent_cache_min_compile_time_secs", 0.0)
except Exception:
    pass

try:
    # bass_utils imports this when tracing is requested via BASS_TRACE; the
    # agent image ships a stub antenv without it. Register a no-op fallback
    # so a trace request degrades to "no trace" instead of crashing.
    from antenv import axon_hooks as _axon_hooks  # noqa: F401
except ImportError:
    import types as _types

    import antenv as _antenv

    _hooks = _types.ModuleType("antenv.axon_hooks")
    _hooks._hook = None
    _hooks.set_axon_ntff_profile_hook = lambda h: setattr(_hooks, "_hook", h)
    _hooks.get_axon_ntff_profile_hook = lambda: _hooks._hook
    sys.modules["antenv.axon_hooks"] = _hooks
    _antenv.axon_hooks = _hooks

import ml_dtypes  # noqa: E402

import concourse.bass as bass  # noqa: E402,F401
import concourse.mybir as mybir  # noqa: E402
import concourse.tile as tile  # noqa: E402
from concourse import bacc  # noqa: E402
from concourse.bass_utils import run_bass_kernel_spmd  # noqa: E402

N_CORES = 8
N_TOK, D_IN, D_OUT = 16384, 1024, 1024
N_SHARD = N_TOK // N_CORES  # 2048 tokens per core
P = 128
SCALING = 4.0  # alpha / r = 32 / 8

BF16 = ml_dtypes.bfloat16

_CACHE: dict = {}


def build_nc():
    f32 = mybir.dt.float32
    bf16 = mybir.dt.bfloat16
    nc = bacc.Bacc("TRN2", target_bir_lowering=False, debug=False)

    xT = nc.dram_tensor("xT", [D_IN, N_SHARD], bf16, kind="ExternalInput")
    wT = nc.dram_tensor("wT", [D_IN, D_OUT], bf16, kind="ExternalInput")
    out = nc.dram_tensor("out", [N_SHARD, D_OUT], bf16, kind="ExternalOutput")

    KT = D_IN // P  # 8 contraction tiles
    NBLK = 512  # tokens per group (4 psum tiles of 128)
    GRP = NBLK // P  # 4 psum tiles accumulated concurrently (8 banks)
    OH = 512  # max fp32 moving free dim (one PSUM bank)

    NGRP = N_SHARD // NBLK
    XBLK = 2 * NBLK  # 1024 tokens per x tile, serves 2 groups
    with tile.TileContext(nc) as tc:
        with tc.tile_pool(name="const", bufs=1) as const_pool, \
                tc.tile_pool(name="xp", bufs=2 * KT) as x_pool, \
                tc.tile_pool(name="op", bufs=8) as out_pool, \
                tc.tile_pool(name="ps", bufs=GRP, space="PSUM") as psum_pool:
            w_tiles = [
                const_pool.tile([P, D_OUT], bf16, name=f"w{t}")
                for t in range(KT)
            ]

            # Leading warm-up matmuls (zeroed bf16 scratch, no data deps).
            warm_x = const_pool.tile([P, P], bf16)
            warm_w = const_pool.tile([P, OH], bf16)
            nc.vector.memset(warm_x[:], 0.0)
            nc.vector.memset(warm_w[:], 0.0)
            warm_ps = psum_pool.tile([P, OH], f32, name="warm_ps", tag="psum")
            for _ in range(6):
                nc.tensor.matmul(warm_ps[:], warm_x[:], warm_w[:],
                                 start=True, stop=True)

            # Startup streams in exact consumption order, on two concurrent
            # HWDGE rings. The first-needed chunks (x0 quarter, then all of
            # w0) ride the sync ring, which kicks off ~0.6us before the
            # scalar ring; the scalar ring brings w1..w7 concurrently. Tile
            # dependencies are sub-tile-range granular, so x tiles load in
            # halves: group 0 only needs token-half 0 of each d-slice; half 1
            # (group 1) streams after.
            x_super = {}
            xs = [
                [
                    x_pool.tile([P, XBLK], bf16, name=f"x_gg{gg}_d{t}",
                                tag="xd")
                    for t in range(KT)
                ]
                for gg in range(NGRP // 2)
            ]
            x_super[0] = xs[0]
            x_super[1] = xs[1]
            nc.scalar.dma_start(w_tiles[0][:, 0:OH], wT[0:P, 0:OH])
            nc.scalar.dma_start(w_tiles[0][:, OH:D_OUT], wT[0:P, OH:D_OUT])
            for t in range(1, KT):
                nc.scalar.dma_start(w_tiles[t][:], wT[t * P:(t + 1) * P, :])
            nc.sync.dma_start(xs[0][0][:, 0:P], xT[0:P, 0:P])

            nc.sync.dma_start(xs[0][0][:, P:NBLK], xT[0:P, P:NBLK])
            for t in range(1, KT):
                nc.sync.dma_start(
                    xs[0][t][:, 0:NBLK], xT[t * P:(t + 1) * P, 0:NBLK]
                )
            for t in range(KT):
                nc.sync.dma_start(
                    xs[0][t][:, NBLK:XBLK], xT[t * P:(t + 1) * P, NBLK:XBLK]
                )
            for t in range(KT):
                nc.sync.dma_start(
                    xs[1][t][:], xT[t * P:(t + 1) * P, XBLK:2 * XBLK]
                )

            def evict(g, i, psum, split=False):
                # Pure psum->sbuf copy with fp32->bf16 convert; bias is added
                # on the host. Alternate engines so two evictions overlap.
                n0 = g * NBLK + i * P
                o_sb = out_pool.tile([P, D_OUT], bf16)
                if split:
                    # tail: halves on both engines concurrently, DMA per half
                    # on both rings.
                    nc.scalar.copy(o_sb[:, 0:OH], psum[:, 0:OH])
                    nc.vector.tensor_copy(o_sb[:, OH:D_OUT], psum[:, OH:D_OUT])
                    nc.scalar.dma_start(out[n0:n0 + P, 0:OH], o_sb[:, 0:OH])
                    nc.sync.dma_start(out[n0:n0 + P, OH:D_OUT],
                                      o_sb[:, OH:D_OUT])
                else:
                    if i % 2 == 0:
                        nc.scalar.copy(o_sb[:], psum[:])
                    else:
                        nc.vector.tensor_copy(o_sb[:], psum[:])
                    nc.scalar.dma_start(out[n0:n0 + P, :], o_sb[:])

            for g in range(NGRP):
                gg, half = divmod(g, 2)
                xt = [
                    x_super[gg][t][:, half * NBLK:(half + 1) * NBLK]
                    for t in range(KT)
                ]
                psums = [
                    psum_pool.tile([P, D_OUT], f32, name=f"ps_g{g}_{i}",
                                   tag="psum")
                    for i in range(GRP)
                ]
                if g < NGRP - 1:
                    # d-outer for d<7: each arriving (w[d], x[d]) slice
                    # immediately enables 8 matmuls while later slices are in
                    # flight. d=7 goes per-psum-tile with the eviction right
                    # after, staggering evictions across the group tail.
                    for d in range(KT - 1):
                        for i in range(GRP):
                            lhsT = xt[d][:, i * P:(i + 1) * P]
                            for h in range(D_OUT // OH):
                                nc.tensor.matmul(
                                    psums[i][:, h * OH:(h + 1) * OH],
                                    lhsT,
                                    w_tiles[d][:, h * OH:(h + 1) * OH],
                                    start=(d == 0),
                                    stop=False,
                                )
                    d = KT - 1
                    for i in range(GRP):
                        lhsT = xt[d][:, i * P:(i + 1) * P]
                        for h in range(D_OUT // OH):
                            nc.tensor.matmul(
                                psums[i][:, h * OH:(h + 1) * OH],
                                lhsT,
                                w_tiles[d][:, h * OH:(h + 1) * OH],
                                start=False,
                                stop=True,
                            )
                        evict(g, i, psums[i])
                else:
                    # last group, data resident: i-outer spreads psum
                    # completions so the tail isn't 4 serialized evictions.
                    for i in range(GRP):
                        for d in range(KT):
                            lhsT = xt[d][:, i * P:(i + 1) * P]
                            for h in range(D_OUT // OH):
                                nc.tensor.matmul(
                                    psums[i][:, h * OH:(h + 1) * OH],
                                    lhsT,
                                    w_tiles[d][:, h * OH:(h + 1) * OH],
                                    start=(d == 0),
                                    stop=(d == KT - 1),
                                )
                        evict(g, i, psums[i], split=True)

    nc.finalize()
    return nc


def _get_nc():
    if "nc" not in _CACHE:
        _CACHE["nc"] = build_nc()
    return _CACHE["nc"]


def kernel(x, weight, bias, A, B):
    x = np.asarray(x, dtype=np.float32)
    weight = np.asarray(weight, dtype=np.float32)
    bias = np.asarray(bias, dtype=np.float32)
    A = np.asarray(A, dtype=np.float32)
    B = np.asarray(B, dtype=np.float32)

    # Fold the rank-8 LoRA update into the weight (exact up to fp32 rounding).
    w_eff = (
        weight.astype(np.float64) + SCALING * (B.astype(np.float64) @ A.astype(np.float64))
    ).astype(np.float32)
    wT = np.ascontiguousarray(w_eff.T).astype(BF16)  # [d, o]
    xT = np.ascontiguousarray(x.T).astype(BF16)  # [d, n]

    nc = _get_nc()
    in_maps = [
        {
            "xT": np.ascontiguousarray(xT[:, c * N_SHARD:(c + 1) * N_SHARD]),
            "wT": wT,
        }
        for c in range(N_CORES)
    ]
    trace_kwargs = {}
    if os.environ.get("KERNEL_TRACE") == "1":
        trace_kwargs = {"trace": True}
    res = run_bass_kernel_spmd(nc, in_maps, list(range(N_CORES)), **trace_kwargs)
    _CACHE["last_results"] = res
    y = np.concatenate(
        [r["out"] for r in res.results], axis=0
    ).astype(np.float32)
    y += bias[None, :]
    return y


# revision 14
# speedup vs baseline: 1.0599x; 1.0025x over previous
"""LoRA linear (y = x @ (W + s*B@A)^T + bias) on 8 Trainium2 NeuronCores.

Strategy: pure data parallel over the token dim. The LoRA update is folded
into the weight on the host (W' = W + 4.0 * B @ A, rank-8 update), and the
bias is added on the host after the device matmul, so the device kernel is a
pure GEMM. All matmul operands are bf16 (fro rel-err ~2.9e-3 vs the 2e-2
gate): halves HBM traffic vs fp32 (20.9 -> 10.5 MB/core) and lets the PE
hide LDWEIGHTS entirely (measured: warm N=512 matmuls issue back-to-back at
216ns with the per-matmul weight load overlapped, i.e. the PE runs at its
streaming roofline). PSUM accumulation stays fp32; the output is stored bf16
and upcast to fp32 on the host.

Per core: out[2048, 1024] = xT[:, shard].T @ wT
  - wT [1024(d), 1024(o)] resident in SBUF (2 MiB), loaded once on the
    scalar-engine HWDGE ring; x streams on the sync-engine ring in exact
    consumption order (the two rings run concurrently)
  - the SDMA rings only start streaming ~2.2us after the sequencers start,
    so real operands land from ~8.2us; five warm-up matmuls on zeroed scratch
    (memset by the vector engine, which starts earliest) occupy the PE's
    HAM clock-gate window from ~7us so the real matmul stream runs at the
    warm 2.4 GHz clock almost immediately
  - psum [128(n), 1024(o)] accumulated over 8 d-tiles, 2 o-halves of 512;
    4 psum tiles (all 8 banks) accumulate side by side so each arriving
    (w[d], x[d]) slice enables 8 matmuls during the fill phase
  - d=7 (the accumulation-closing matmuls) is issued per-psum-tile with the
    eviction immediately after, so the 4 evictions of a group stagger
    across the group tail instead of bunching at the boundary
  - evictions are pure fp32->bf16 copies alternating between the scalar
    (ACT) and vector (DVE) engines so two run concurrently; out DMAs ride
    the scalar ring (empty after the w fill), and the last group closes
    i-at-a-time with half-evictions on both engines and half-DMAs on both
    rings to shorten the tail
"""

import os
import sys

import numpy as np

for _p in ("/opt/trn_rl_repo", "/opt/pypackages"):
    if os.path.isdir(_p) and _p not in sys.path:
        sys.path.append(_p)

try:
    import jax

    jax.config.update(
        "jax_compilation_cache_dir", os.path.expanduser("~/.cache/jax_bass_cache")
    )
    jax.config.update("jax_persistent_cache_min_compile_time_secs", 0.0)
except Exception:
    pass

try:
    # bass_utils imports this when tracing is requested via BASS_TRACE; the
    # agent image ships a stub antenv without it. Register a no-op fallback
    # so a trace request degrades to "no trace" instead of crashing.
    from antenv import axon_hooks as _axon_hooks  # noqa: F401
except ImportError:
    import types as _types

    import antenv as _antenv

    _hooks = _types.ModuleType("antenv.axon_hooks")
    _hooks._hook = None
    _hooks.set_axon_ntff_profile_hook = lambda h: setattr(_hooks, "_hook", h)
    _hooks.get_axon_ntff_profile_hook = lambda: _hooks._hook
    sys.modules["antenv.axon_hooks"] = _hooks
    _antenv.axon_hooks = _hooks

import ml_dtypes  # noqa: E402

import concourse.bass as bass  # noqa: E402,F401
import concourse.mybir as mybir  # noqa: E402
import concourse.tile as tile  # noqa: E402
from concourse import bacc  # noqa: E402
from concourse.bass_utils import run_bass_kernel_spmd  # noqa: E402

N_CORES = 8
N_TOK, D_IN, D_OUT = 16384, 1024, 1024
N_SHARD = N_TOK // N_CORES  # 2048 tokens per core
P = 128
SCALING = 4.0  # alpha / r = 32 / 8

BF16 = ml_dtypes.bfloat16

_CACHE: dict = {}


def build_nc():
    f32 = mybir.dt.float32
    bf16 = mybir.dt.bfloat16
    nc = bacc.Bacc("TRN2", target_bir_lowering=False, debug=False)

    xT = nc.dram_tensor("xT", [D_IN, N_SHARD], bf16, kind="ExternalInput")
    wT = nc.dram_tensor("wT", [D_IN, D_OUT], bf16, kind="ExternalInput")
    out = nc.dram_tensor("out", [N_SHARD, D_OUT], bf16, kind="ExternalOutput")

    KT = D_IN // P  # 8 contraction tiles
    NBLK = 512  # tokens per group (4 psum tiles of 128)
    GRP = NBLK // P  # 4 psum tiles accumulated concurrently (8 banks)
    OH = 512  # max fp32 moving free dim (one PSUM bank)

    NGRP = N_SHARD // NBLK
    XBLK = 2 * NBLK  # 1024 tokens per x tile, serves 2 groups
    with tile.TileContext(nc) as tc:
        with tc.tile_pool(name="const", bufs=1) as const_pool, \
                tc.tile_pool(name="xp", bufs=2 * KT) as x_pool, \
                tc.tile_pool(name="op", bufs=8) as out_pool, \
                tc.tile_pool(name="ps", bufs=GRP, space="PSUM") as psum_pool:
            w_tiles = [
                const_pool.tile([P, D_OUT], bf16, name=f"w{t}")
                for t in range(KT)
            ]

            # Leading warm-up matmuls (zeroed bf16 scratch, no data deps).
            warm_x = const_pool.tile([P, P], bf16)
            warm_w = const_pool.tile([P, OH], bf16)
            nc.vector.memset(warm_x[:], 0.0)
            nc.vector.memset(warm_w[:], 0.0)
            warm_ps = psum_pool.tile([P, OH], f32, name="warm_ps", tag="psum")
            for _ in range(5):
                nc.tensor.matmul(warm_ps[:], warm_x[:], warm_w[:],
                                 start=True, stop=True)

            # Startup streams in exact consumption order, on two concurrent
            # HWDGE rings. The first-needed chunks (x0 quarter, then all of
            # w0) ride the sync ring, which kicks off ~0.6us before the
            # scalar ring; the scalar ring brings w1..w7 concurrently. Tile
            # dependencies are sub-tile-range granular, so x tiles load in
            # halves: group 0 only needs token-half 0 of each d-slice; half 1
            # (group 1) streams after.
            x_super = {}
            xs = [
                [
                    x_pool.tile([P, XBLK], bf16, name=f"x_gg{gg}_d{t}",
                                tag="xd")
                    for t in range(KT)
                ]
                for gg in range(NGRP // 2)
            ]
            x_super[0] = xs[0]
            x_super[1] = xs[1]
            nc.scalar.dma_start(w_tiles[0][:, 0:OH], wT[0:P, 0:OH])
            nc.scalar.dma_start(w_tiles[0][:, OH:D_OUT], wT[0:P, OH:D_OUT])
            for t in range(1, KT):
                nc.scalar.dma_start(w_tiles[t][:], wT[t * P:(t + 1) * P, :])
            nc.sync.dma_start(xs[0][0][:, 0:P], xT[0:P, 0:P])

            nc.sync.dma_start(xs[0][0][:, P:NBLK], xT[0:P, P:NBLK])
            for t in range(1, KT):
                nc.sync.dma_start(
                    xs[0][t][:, 0:NBLK], xT[t * P:(t + 1) * P, 0:NBLK]
                )
            for t in range(KT):
                nc.sync.dma_start(
                    xs[0][t][:, NBLK:XBLK], xT[t * P:(t + 1) * P, NBLK:XBLK]
                )
            for t in range(KT):
                nc.sync.dma_start(
                    xs[1][t][:], xT[t * P:(t + 1) * P, XBLK:2 * XBLK]
                )

            def evict(g, i, psum, split=False):
                # Pure psum->sbuf copy with fp32->bf16 convert; bias is added
                # on the host. Alternate engines so two evictions overlap.
                n0 = g * NBLK + i * P
                o_sb = out_pool.tile([P, D_OUT], bf16)
                if split:
                    # tail: halves on both engines concurrently, DMA per half
                    # on both rings.
                    nc.scalar.copy(o_sb[:, 0:OH], psum[:, 0:OH])
                    nc.vector.tensor_copy(o_sb[:, OH:D_OUT], psum[:, OH:D_OUT])
                    nc.scalar.dma_start(out[n0:n0 + P, 0:OH], o_sb[:, 0:OH])
                    nc.sync.dma_start(out[n0:n0 + P, OH:D_OUT],
                                      o_sb[:, OH:D_OUT])
                else:
                    if i % 2 == 0:
                        nc.scalar.copy(o_sb[:], psum[:])
                    else:
                        nc.vector.tensor_copy(o_sb[:], psum[:])
                    nc.scalar.dma_start(out[n0:n0 + P, :], o_sb[:])

            for g in range(NGRP):
                gg, half = divmod(g, 2)
                xt = [
                    x_super[gg][t][:, half * NBLK:(half + 1) * NBLK]
                    for t in range(KT)
                ]
                psums = [
                    psum_pool.tile([P, D_OUT], f32, name=f"ps_g{g}_{i}",
                                   tag="psum")
                    for i in range(GRP)
                ]
                if g < NGRP - 1:
                    # d-outer for d<7: each arriving (w[d], x[d]) slice
                    # immediately enables 8 matmuls while later slices are in
                    # flight. d=7 goes per-psum-tile with the eviction right
                    # after, staggering evictions across the group tail.
                    for d in range(KT - 1):
                        for i in range(GRP):
                            lhsT = xt[d][:, i * P:(i + 1) * P]
                            for h in range(D_OUT // OH):
                                nc.tensor.matmul(
                                    psums[i][:, h * OH:(h + 1) * OH],
                                    lhsT,
                                    w_tiles[d][:, h * OH:(h + 1) * OH],
                                    start=(d == 0),
                                    stop=False,
                                )
                    d = KT - 1
                    for i in range(GRP):
                        lhsT = xt[d][:, i * P:(i + 1) * P]
                        for h in range(D_OUT // OH):
                            nc.tensor.matmul(
                                psums[i][:, h * OH:(h + 1) * OH],
                                lhsT,
                                w_tiles[d][:, h * OH:(h + 1) * OH],
                                start=False,
                                stop=True,
                            )
                        evict(g, i, psums[i])
                else:
                    # last group, data resident: i-outer spreads psum
                    # completions so the tail isn't 4 serialized evictions.
                    for i in range(GRP):
                        for d in range(KT):
                            lhsT = xt[d][:, i * P:(i + 1) * P]
                            for h in range(D_OUT // OH):
                                nc.tensor.matmul(
                                    psums[i][:, h * OH:(h + 1) * OH],
                                    lhsT,
                                    w_tiles[d][:, h * OH:(h + 1) * OH],
                                    start=(d == 0),
                                    stop=(d == KT - 1),
                                )
                        evict(g, i, psums[i], split=True)

    nc.finalize()
    return nc


def _get_nc():
    if "nc" not in _CACHE:
        _CACHE["nc"] = build_nc()
    return _CACHE["nc"]


def kernel(x, weight, bias, A, B):
    x = np.asarray(x, dtype=np.float32)
    weight = np.asarray(weight, dtype=np.float32)
    bias = np.asarray(bias, dtype=np.float32)
    A = np.asarray(A, dtype=np.float32)
    B = np.asarray(B, dtype=np.float32)

    # Fold the rank-8 LoRA update into the weight (exact up to fp32 rounding).
    w_eff = (
        weight.astype(np.float64) + SCALING * (B.astype(np.float64) @ A.astype(np.float64))
    ).astype(np.float32)
    wT = np.ascontiguousarray(w_eff.T).astype(BF16)  # [d, o]
    xT = np.ascontiguousarray(x.T).astype(BF16)  # [d, n]

    nc = _get_nc()
    in_maps = [
        {
            "xT": np.ascontiguousarray(xT[:, c * N_SHARD:(c + 1) * N_SHARD]),
            "wT": wT,
        }
        for c in range(N_CORES)
    ]
    trace_kwargs = {}
    if os.environ.get("KERNEL_TRACE") == "1":
        trace_kwargs = {"trace": True}
    res = run_bass_kernel_spmd(nc, in_maps, list(range(N_CORES)), **trace_kwargs)
    _CACHE["last_results"] = res
    y = np.concatenate(
        [r["out"] for r in res.results], axis=0
    ).astype(np.float32)
    y += bias[None, :]
    return y


# revision 16
# speedup vs baseline: 1.0770x; 1.0162x over previous
"""LoRA linear (y = x @ (W + s*B@A)^T + bias) on 8 Trainium2 NeuronCores.

Strategy: pure data parallel over the token dim. The LoRA update is folded
into the weight on the host (W' = W + 4.0 * B @ A, rank-8 update), and the
bias is added on the host after the device matmul, so the device kernel is a
pure GEMM. All matmul operands are bf16 (fro rel-err ~2.9e-3 vs the 2e-2
gate): halves HBM traffic vs fp32 (20.9 -> 10.5 MB/core) and enables the PE
fast-weight-load path (FWL, non-fp32 only). PSUM accumulation stays fp32;
the output is stored bf16 and upcast to fp32 on the host.

Per core: out[2048, 1024] = xT[:, shard].T @ wT
  - wT [1024(d), 1024(o)] resident in SBUF (2 MiB), loaded once, streamed on
    the scalar-engine HWDGE ring (qActDynamicHW)
  - x resident as 8 d-tiles x 2 super-blocks of 1024 tokens, streamed on the
    sync-engine HWDGE ring (qSPDynamicHW) in exact consumption order; the
    two rings run concurrently so the startup fill is ~2x faster
  - psum [128(n), 1024(o)] accumulated over 8 d-tiles, 2 o-halves of 512;
    4 psum tiles (all 8 banks) accumulate side by side. Group 0 (the fill
    phase) runs d-outer for d0..d5 so each arriving (w[d], x[d]) slice
    enables 8 matmuls, then closes per-psum-tile over d6+d7 with the
    eviction right after; w5..w7 ride the sync ring behind the group-0 x
    halves so neither ring straggles during the ramp. The resident middle
    groups run i-outer so each eviction hides under ~3.5us of the next
    psum's matmuls — boundary stalls vanish
  - evictions are pure fp32->bf16 copies, alternating between the scalar
    (ACT) and vector (DVE) engines so two run concurrently
  - out DMAs ride the scalar ring (empty after the w fill)
"""

import os
import sys

import numpy as np

for _p in ("/opt/trn_rl_repo", "/opt/pypackages"):
    if os.path.isdir(_p) and _p not in sys.path:
        sys.path.append(_p)

try:
    import jax

    jax.config.update(
        "jax_compilation_cache_dir", os.path.expanduser("~/.cache/jax_bass_cache")
    )
    jax.config.update("jax_persistent_cache_min_compile_time_secs", 0.0)
except Exception:
    pass

try:
    # bass_utils imports this when tracing is requested via BASS_TRACE; the
    # agent image ships a stub antenv without it. Register a no-op fallback
    # so a trace request degrades to "no trace" instead of crashing.
    from antenv import axon_hooks as _axon_hooks  # noqa: F401
except ImportError:
    import types as _types

    import antenv as _antenv

    _hooks = _types.ModuleType("antenv.axon_hooks")
    _hooks._hook = None
    _hooks.set_axon_ntff_profile_hook = lambda h: setattr(_hooks, "_hook", h)
    _hooks.get_axon_ntff_profile_hook = lambda: _hooks._hook
    sys.modules["antenv.axon_hooks"] = _hooks
    _antenv.axon_hooks = _hooks

import ml_dtypes  # noqa: E402

import concourse.bass as bass  # noqa: E402,F401
import concourse.mybir as mybir  # noqa: E402
import concourse.tile as tile  # noqa: E402
from concourse import bacc  # noqa: E402
from concourse.bass_utils import run_bass_kernel_spmd  # noqa: E402

N_CORES = 8
N_TOK, D_IN, D_OUT = 16384, 1024, 1024
N_SHARD = N_TOK // N_CORES  # 2048 tokens per core
P = 128
SCALING = 4.0  # alpha / r = 32 / 8

BF16 = ml_dtypes.bfloat16

_CACHE: dict = {}


def build_nc():
    f32 = mybir.dt.float32
    bf16 = mybir.dt.bfloat16
    nc = bacc.Bacc("TRN2", target_bir_lowering=False, debug=False)

    xT = nc.dram_tensor("xT", [D_IN, N_SHARD], bf16, kind="ExternalInput")
    wT = nc.dram_tensor("wT", [D_IN, D_OUT], bf16, kind="ExternalInput")
    out = nc.dram_tensor("out", [N_SHARD, D_OUT], bf16, kind="ExternalOutput")

    KT = D_IN // P  # 8 contraction tiles
    NBLK = 512  # tokens per group (4 psum tiles of 128)
    GRP = NBLK // P  # 4 psum tiles accumulated concurrently (8 banks)
    OH = 512  # max fp32 moving free dim (one PSUM bank)

    NGRP = N_SHARD // NBLK
    XBLK = 2 * NBLK  # 1024 tokens per x tile, serves 2 groups
    with tile.TileContext(nc) as tc:
        with tc.tile_pool(name="const", bufs=1) as const_pool, \
                tc.tile_pool(name="xp", bufs=2 * KT) as x_pool, \
                tc.tile_pool(name="op", bufs=8) as out_pool, \
                tc.tile_pool(name="ps", bufs=GRP, space="PSUM") as psum_pool:
            w_tiles = [
                const_pool.tile([P, D_OUT], bf16, name=f"w{t}")
                for t in range(KT)
            ]

            # Leading warm-up matmuls (zeroed bf16 scratch, no data deps).
            warm_x = const_pool.tile([P, P], bf16)
            warm_w = const_pool.tile([P, OH], bf16)
            nc.vector.memset(warm_x[:], 0.0)
            nc.vector.memset(warm_w[:], 0.0)
            warm_ps = psum_pool.tile([P, OH], f32, name="warm_ps", tag="psum")
            for _ in range(6):
                nc.tensor.matmul(warm_ps[:], warm_x[:], warm_w[:],
                                 start=True, stop=True)

            # Startup streams in exact consumption order, on two concurrent
            # HWDGE rings. The first-needed chunks (x0 quarter, then all of
            # w0) ride the sync ring, which kicks off ~0.6us before the
            # scalar ring; the scalar ring brings w1..w7 concurrently. Tile
            # dependencies are sub-tile-range granular, so x tiles load in
            # halves: group 0 only needs token-half 0 of each d-slice; half 1
            # (group 1) streams after.
            x_super = {}
            xs = [
                [
                    x_pool.tile([P, XBLK], bf16, name=f"x_gg{gg}_d{t}",
                                tag="xd")
                    for t in range(KT)
                ]
                for gg in range(NGRP // 2)
            ]
            x_super[0] = xs[0]
            x_super[1] = xs[1]
            nc.scalar.dma_start(w_tiles[0][:, 0:OH], wT[0:P, 0:OH])
            nc.scalar.dma_start(w_tiles[0][:, OH:D_OUT], wT[0:P, OH:D_OUT])
            for t in range(1, KT - 3):
                nc.scalar.dma_start(w_tiles[t][:], wT[t * P:(t + 1) * P, :])
            nc.sync.dma_start(xs[0][0][:, 0:P], xT[0:P, 0:P])

            nc.sync.dma_start(xs[0][0][:, P:NBLK], xT[0:P, P:NBLK])
            for t in range(1, KT):
                nc.sync.dma_start(
                    xs[0][t][:, 0:NBLK], xT[t * P:(t + 1) * P, 0:NBLK]
                )
            for t in range(KT - 3, KT):
                nc.sync.dma_start(w_tiles[t][:], wT[t * P:(t + 1) * P, :])
            for t in range(KT):
                nc.sync.dma_start(
                    xs[0][t][:, NBLK:XBLK], xT[t * P:(t + 1) * P, NBLK:XBLK]
                )
            for t in range(KT):
                nc.sync.dma_start(
                    xs[1][t][:], xT[t * P:(t + 1) * P, XBLK:2 * XBLK]
                )

            def evict(g, i, psum, split=False):
                # Pure psum->sbuf copy with fp32->bf16 convert; bias is added
                # on the host. Alternate engines so two evictions overlap.
                n0 = g * NBLK + i * P
                o_sb = out_pool.tile([P, D_OUT], bf16)
                if split:
                    # tail: halves on both engines concurrently, DMA per half
                    # on both rings.
                    nc.scalar.copy(o_sb[:, 0:OH], psum[:, 0:OH])
                    nc.vector.tensor_copy(o_sb[:, OH:D_OUT], psum[:, OH:D_OUT])
                    nc.scalar.dma_start(out[n0:n0 + P, 0:OH], o_sb[:, 0:OH])
                    nc.sync.dma_start(out[n0:n0 + P, OH:D_OUT],
                                      o_sb[:, OH:D_OUT])
                else:
                    if i % 2 == 0:
                        nc.scalar.copy(o_sb[:], psum[:])
                    else:
                        nc.vector.tensor_copy(o_sb[:], psum[:])
                    nc.scalar.dma_start(out[n0:n0 + P, :], o_sb[:])

            for g in range(NGRP):
                gg, half = divmod(g, 2)
                xt = [
                    x_super[gg][t][:, half * NBLK:(half + 1) * NBLK]
                    for t in range(KT)
                ]
                psums = [
                    psum_pool.tile([P, D_OUT], f32, name=f"ps_g{g}_{i}",
                                   tag="psum")
                    for i in range(GRP)
                ]
                if g == 0:
                    # Fill phase: d-outer for d0..d5 so each arriving
                    # (w[d], x[d]) slice immediately enables 8 matmuls. The
                    # last two d-steps run per-psum-tile with the eviction
                    # right after, closing psums ~860ns apart so the
                    # evictions finish before group 1 reuses the banks.
                    for d in range(KT - 2):
                        for i in range(GRP):
                            lhsT = xt[d][:, i * P:(i + 1) * P]
                            for h in range(D_OUT // OH):
                                nc.tensor.matmul(
                                    psums[i][:, h * OH:(h + 1) * OH],
                                    lhsT,
                                    w_tiles[d][:, h * OH:(h + 1) * OH],
                                    start=(d == 0),
                                    stop=False,
                                )
                    for i in range(GRP):
                        for d in (KT - 2, KT - 1):
                            lhsT = xt[d][:, i * P:(i + 1) * P]
                            for h in range(D_OUT // OH):
                                nc.tensor.matmul(
                                    psums[i][:, h * OH:(h + 1) * OH],
                                    lhsT,
                                    w_tiles[d][:, h * OH:(h + 1) * OH],
                                    start=False,
                                    stop=(d == KT - 1),
                                )
                        evict(g, i, psums[i])
                elif g < NGRP - 1:
                    # data resident: i-outer gives each eviction ~3.5us of
                    # slack under the next psum's matmuls, so boundary
                    # stalls vanish.
                    for i in range(GRP):
                        for d in range(KT):
                            lhsT = xt[d][:, i * P:(i + 1) * P]
                            for h in range(D_OUT // OH):
                                nc.tensor.matmul(
                                    psums[i][:, h * OH:(h + 1) * OH],
                                    lhsT,
                                    w_tiles[d][:, h * OH:(h + 1) * OH],
                                    start=(d == 0),
                                    stop=(d == KT - 1),
                                )
                        evict(g, i, psums[i])
                else:
                    # last group, data resident: i-outer spreads psum
                    # completions so the tail isn't 4 serialized evictions.
                    for i in range(GRP):
                        for d in range(KT):
                            lhsT = xt[d][:, i * P:(i + 1) * P]
                            for h in range(D_OUT // OH):
                                nc.tensor.matmul(
                                    psums[i][:, h * OH:(h + 1) * OH],
                                    lhsT,
                                    w_tiles[d][:, h * OH:(h + 1) * OH],
                                    start=(d == 0),
                                    stop=(d == KT - 1),
                                )
                        evict(g, i, psums[i], split=True)

    nc.finalize()
    return nc


def _get_nc():
    if "nc" not in _CACHE:
        _CACHE["nc"] = build_nc()
    return _CACHE["nc"]


def kernel(x, weight, bias, A, B):
    x = np.asarray(x, dtype=np.float32)
    weight = np.asarray(weight, dtype=np.float32)
    bias = np.asarray(bias, dtype=np.float32)
    A = np.asarray(A, dtype=np.float32)
    B = np.asarray(B, dtype=np.float32)

    # Fold the rank-8 LoRA update into the weight (exact up to fp32 rounding).
    w_eff = (
        weight.astype(np.float64) + SCALING * (B.astype(np.float64) @ A.astype(np.float64))
    ).astype(np.float32)
    wT = np.ascontiguousarray(w_eff.T).astype(BF16)  # [d, o]
    xT = np.ascontiguousarray(x.T).astype(BF16)  # [d, n]

    nc = _get_nc()
    in_maps = [
        {
            "xT": np.ascontiguousarray(xT[:, c * N_SHARD:(c + 1) * N_SHARD]),
            "wT": wT,
        }
        for c in range(N_CORES)
    ]
    trace_kwargs = {}
    if os.environ.get("KERNEL_TRACE") == "1":
        trace_kwargs = {"trace": True}
    res = run_bass_kernel_spmd(nc, in_maps, list(range(N_CORES)), **trace_kwargs)
    _CACHE["last_results"] = res
    y = np.concatenate(
        [r["out"] for r in res.results], axis=0
    ).astype(np.float32)
    y += bias[None, :]
    return y
